# revision 28
# baseline (speedup 1.0000x reference)
"""Trainium2 Bass kernel for nn_MemoryEfficientAttention (full MHA).

Reference computation (fp32):
    q = split_heads(x @ Wq.T + bq); k, v likewise
    attn = softmax(q @ k.T / sqrt(64))
    out = merge_heads(attn @ v) @ Wo.T + bo

Shapes: B=2, S=4096, D=1024, H=16, head_dim=64.

Sharding across 8 NeuronCores (Megatron attention sharding):
  - 2 heads per core (= 128 of the 1024 projection dims, contiguous slice).
  - Q/K/V projections column-parallel, output projection row-parallel;
    the 8 per-core partial outputs are summed on the host (+ bo).
  - bv never enters the device: softmax rows sum to 1, so its entire effect
    on the output is the constant vector Wo @ bv, added on the host.

Per-core kernel (flash-attention style, nothing S^2-sized touches HBM):
  Phase 1: qT/kT = W_c @ x.T + b_c as fp16 matmuls ([128, S] transposed
           layouts); V projected directly in natural [S, 128] layout
           (x-tile stationary), stored in fp8 pair slabs with an
           interleaved ones-column per head ([vA|1|pad|vB|1]) that makes
           the PV matmul accumulate the softmax denominator in PSUM row
           64 for free.
  Phase 2: per (batch, q-chunk of 512): loop over 32 key tiles:
           scoresT[kpos, q] for both heads (row-packed in the PE array,
           they run concurrently), exp split between ScalarE (exact) and
           VectorE (Schraudolph fp8 bit-trick exp); scores run several
           key tiles ahead of the PV matmuls, which are emitted in bursts
           of 2 pairs so score-pair runs keep LDWEIGHTS pipelining. PV
           runs in fp8e4 DoubleRow: one matmul per head contracts a
           key-tile PAIR (256 virtual rows). At chunk end the denominator
           row is copied out, reciprocal'd (DVE), and broadcast across 64
           partitions on the otherwise-idle GpSimd engine
           (partition_broadcast); the PSUM release copy is FUSED with
           normalization (tensor_mul acc*rcp_bcast -> attT f16), deferred
           into the next chunk's ACT-only window. Out-projection pieces
           of the previous chunk are spread one-per-3-key-tiles.
  Phase 3: out[s, :] = attnT_c.T @ WoT_c (natural layout, clean DMA out).
  Startup: batch-0 projections are interleaved INTO q-chunk 0's key-tile
           loop (proj chunk m feeds key tiles 4m..4m+3), so ACT starts
           exp work ~4us in instead of after the whole projection phase.
"""

import sys

if "/opt/trn_rl_repo" not in sys.path:
    sys.path.insert(0, "/opt/trn_rl_repo")

import numpy as np

B = 2
S_FULL = 4096
D = 1024
H = 16
HD = 64
NCORES = 8
DC = 128          # head dims per core (2 heads x 64)
SCALE = 1.0 / 8.0  # 1/sqrt(64)

# --- softmax-exp engine split (columns of each [128, 1024] scores tile) ---
# ACT does exact exp; DVE computes a Schraudolph fp8 exp
# (i8 = floor(8*(x*SCALE*log2e + 7 + SIG) + 0.5), bit-viewed as f8e4).
# The flat half-LSB shift cancels in softmax normalization; the residual
# piecewise-linear bias (~3% max) costs ~5e-3 total rel err.
EXP_ACT = 768
EXP_DVE = 1024 - EXP_ACT
EXP_FULL_J = 4    # first key tiles per chunk: ACT does all 1024 cols, giving
                  # DVE room to clear its chunk-boundary burst off the PV path
RELEASE_J = 2     # key-tile index at which the previous chunk's deferred
                  # fused release+normalize muls are emitted on DVE
TAIL_JS = (7, 10, 13, 16, 19, 22, 25, 28)  # key-tile indices at which the
                  # previous chunk's out-projection pieces are emitted
SCH8_SIG = -0.046
SCH8_C1 = float(8.0 * np.log2(np.e) * SCALE)
SCH8_C2 = float(8.0 * (7.0 + SCH8_SIG) + 0.5)


def build_kernel(S=S_FULL):
    """Build the per-core Bass program. Returns the compiled Bacc object."""
    import concourse.bacc as bacc
    import concourse.tile as tile
    from concourse import mybir

    f32 = mybir.dt.float32
    f16 = mybir.dt.float16
    AF = mybir.ActivationFunctionType

    KT = D // 128       # k-tiles over the projection contraction dim
    SQ = 512            # q-chunk size
    NQC = S // SQ       # q chunks per batch
    NKT = S // 128      # key tiles per batch
    NM = S // 512       # x chunks for projections

    nc = bacc.Bacc("TRN2", target_bir_lowering=False, debug=False,
                   num_devices=NCORES)

    u8 = mybir.dt.uint8
    f8 = mybir.dt.float8e4

    xT = nc.dram_tensor("xT", [B, D, S], f16, kind="ExternalInput").ap()
    wqT = nc.dram_tensor("wqT", [D, DC], f16, kind="ExternalInput").ap()
    wkT = nc.dram_tensor("wkT", [D, DC], f16, kind="ExternalInput").ap()
    wvT = nc.dram_tensor("wvT", [D, DC], f16, kind="ExternalInput").ap()
    woT = nc.dram_tensor("woT", [DC, D], f16, kind="ExternalInput").ap()
    bq = nc.dram_tensor("bq", [DC], f32, kind="ExternalInput").ap()
    bk = nc.dram_tensor("bk", [DC], f32, kind="ExternalInput").ap()
    ones = nc.dram_tensor("ones", [128, 32], f16, kind="ExternalInput").ap()
    part = nc.dram_tensor("part", [B, S, D], f16, kind="ExternalOutput").ap()

    with tile.TileContext(nc) as tc:
        with (
            tc.tile_pool(name="consts", bufs=1) as consts,
            tc.tile_pool(name="xt", bufs=5) as xt_pool,
            tc.tile_pool(name="qkv", bufs=2) as qkv_pool,
            tc.tile_pool(name="exp", bufs=4) as exp_pool,
            tc.tile_pool(name="att", bufs=2) as att_pool,
            tc.tile_pool(name="small", bufs=4) as small_pool,
            tc.tile_pool(name="outs", bufs=6) as out_pool,
            tc.tile_pool(name="ps_mix", bufs=2, space="PSUM") as ps_mix,
            tc.tile_pool(name="ps_scores", bufs=2, space="PSUM") as ps_scores,
            tc.tile_pool(name="ps_acc", bufs=2, space="PSUM") as ps_acc,
        ):
            # ---- constants ----
            wq_sb = consts.tile([128, KT, DC], f16)
            wk_sb = consts.tile([128, KT, DC], f16)
            wv_sb = consts.tile([128, KT, DC], f16)
            wo_sb = consts.tile([128, D], f16)
            bq_sb = consts.tile([128, 1], f32)
            bk_sb = consts.tile([128, 1], f32)

            # tiny dummy exp so the ACT table set loads during startup DMAs
            warm = consts.tile([128, 1], f32)
            nc.vector.memset(warm[:], 0.0)
            nc.scalar.activation(warm[:], warm[:], AF.Exp, scale=1.0)

            for w_sb, w_dram in ((wq_sb, wqT), (wk_sb, wkT), (wv_sb, wvT)):
                nc.gpsimd.dma_start(
                    out=w_sb[:],
                    in_=w_dram.rearrange("(kt p) m -> p kt m", p=128),
                )
            nc.gpsimd.dma_start(out=wo_sb[:], in_=woT)
            for b_sb, b_dram in ((bq_sb, bq), (bk_sb, bk)):
                nc.gpsimd.dma_start(out=b_sb[:], in_=b_dram.rearrange("(p o) -> p o", o=1))

            state = [None, None]  # per-batch dict of tiles

            def alloc_batch(b):
                qT_sb = qkv_pool.tile([128, S], f16, tag="qT", name=f"qT_{b}")
                kT_sb = qkv_pool.tile([128, S], f16, tag="kT", name=f"kT_{b}")
                # v in fp8e4 DoubleRow pair layout: per key-tile PAIR jp,
                # slab c = key tile 2jp+c: [vA(64) | 1 | pad | vB(64) | 1]
                # (ones column accumulates the softmax denominator in PSUM
                # row 64 of the PV accumulator for free).
                v_sb = qkv_pool.tile([128, NKT // 2, 2, 160], f8, tag="v",
                                     name=f"v_{b}")
                nc.vector.memset(v_sb[:, :, :, 64:65], 1.0)
                nc.vector.memset(v_sb[:, :, :, 144:145], 1.0)
                attT_sb = att_pool.tile([128, S], f16, tag="attT",
                                        name=f"attT_{b}")
                state[b] = dict(qT=qT_sb, kT=kT_sb, v=v_sb, attT=attT_sb)

            xt_box = {}

            def proj_dma(b, m):
                """Issue the x-chunk load for (b, m) - emitted 1-2 chunks
                ahead of proj_mms so the 1MB DMA never stalls the PE."""
                xt = xt_pool.tile([128, KT, 512], f16, tag="xt",
                                  name=f"xt_{b}_{m}")
                xt_box[(b, m)] = xt
                xsrc = xT[b][:, m * 512:(m + 1) * 512].rearrange(
                    "(kt p) s -> p kt s", p=128)
                third = KT // 3 + 1
                nc.sync.dma_start(out=xt[:, 0:third, :],
                                  in_=xsrc[:, 0:third, :])
                nc.scalar.dma_start(out=xt[:, third:2 * third, :],
                                    in_=xsrc[:, third:2 * third, :])
                nc.gpsimd.dma_start(out=xt[:, 2 * third:, :],
                                    in_=xsrc[:, 2 * third:, :])

            def proj_mms(b, m):
                """Projection matmuls for x columns [m*512, (m+1)*512)."""
                st = state[b]
                xt = xt_box.pop((b, m))
                for w_sb, b_sb, dst in (
                    (wk_sb, bk_sb, st["kT"]),
                    (wq_sb, bq_sb, st["qT"]),
                ):
                    ps = ps_mix.tile([128, 512], f32, tag="mix",
                                      name=f"ps_{b}_{m}")
                    for j in range(KT):
                        nc.tensor.matmul(
                            ps[:],
                            lhsT=w_sb[:, j, :],
                            rhs=xt[:, j, :],
                            start=(j == 0),
                            stop=(j == KT - 1),
                        )
                    nc.vector.tensor_scalar_add(
                        dst[:, m * 512:(m + 1) * 512], ps[:], b_sb[:],
                    )
                # V in natural layout: x-tile stationary, Wv moving.
                for t in range(4):
                    psv = ps_mix.tile([128, 512], f32, tag="mix",
                                       name=f"psv_{b}_{m}_{t}")
                    for j in range(KT):
                        nc.tensor.matmul(
                            psv[:, 0:DC],
                            lhsT=xt[:, j, t * 128:(t + 1) * 128],
                            rhs=wv_sb[:, j, :],
                            start=(j == 0),
                            stop=(j == KT - 1),
                        )
                    kt_idx = m * 4 + t
                    jp, cc = kt_idx // 2, kt_idx % 2
                    with nc.allow_low_precision(reason="fp8 PV operand"):
                        nc.vector.tensor_copy(
                            state[b]["v"][:, jp, cc, 0:64], psv[:, 0:64])
                        nc.vector.tensor_copy(
                            state[b]["v"][:, jp, cc, 80:144], psv[:, 64:128])

            def emit_proj_chunk(b, m):
                proj_dma(b, m)
                proj_mms(b, m)

            def emit_attn(b, qc, fillers=None, all_act=False):
                """Attention for one q-chunk of 512 rows.

                Software-pipelined: scores for key tile j+1 are emitted
                (and thus queued on the in-order PE) BEFORE the PV matmuls
                of tile j, so the PE streams scores while ACT/DVE exp
                catches up. `fillers` is a list of (j, thunk) pairs: thunk
                is emitted when the key-tile loop reaches j (used to
                interleave projection chunks into the attention stream).
                """
                st = state[b]
                qT_sb, kT_sb, v_sb = st["qT"], st["kT"], st["v"]
                attT_sb = st["attT"]
                q0, q1 = qc * SQ, (qc + 1) * SQ
                acc_a = ps_acc.tile([128, SQ], f32, tag="acc",
                                    name=f"acca_{b}_{qc}")
                acc_b = ps_acc.tile([128, SQ], f32, tag="acc",
                                    name=f"accb_{b}_{qc}")
                accs = [acc_a, acc_b]

                if all_act:
                    full_j = NKT
                elif fillers:
                    full_j = EXP_FULL_J + 4
                else:
                    full_j = EXP_FULL_J

                def emit_scores(j, ex8):
                    """Scores pair for key tile j; exp written into slab
                    j%2 of the fp8 pair tile ex8."""
                    k0, k1 = j * 128, (j + 1) * 128
                    pss = ps_scores.tile([128, 2 * SQ], f32, tag="scores",
                                         name=f"pss_{b}_{qc}_{j}")
                    for hh in range(2):
                        nc.tensor.matmul(
                            pss[:, hh * SQ:(hh + 1) * SQ],
                            lhsT=kT_sb[hh * 64:(hh + 1) * 64, k0:k1],
                            rhs=qT_sb[hh * 64:(hh + 1) * 64, q0:q1],
                            start=True, stop=True,
                        )
                    sl = j % 2
                    with nc.allow_low_precision(reason="fp8 softmax weights"):
                        if j < full_j:
                            nc.scalar.activation(ex8[:, sl, :], pss[:],
                                                 AF.Exp, scale=SCALE)
                        else:
                            c0 = EXP_ACT
                            nc.scalar.activation(ex8[:, sl, 0:c0],
                                                 pss[:, 0:c0],
                                                 AF.Exp, scale=SCALE)
                            nc.vector.tensor_scalar(
                                ex8[:, sl, c0:].bitcast(u8), pss[:, c0:],
                                SCH8_C1, SCH8_C2,
                                op0=mybir.AluOpType.mult,
                                op1=mybir.AluOpType.add)

                def emit_pv_half(jp, ex8, hh):
                    """fp8 DoubleRow PV: one matmul per head contracts the
                    key-tile PAIR (2jp, 2jp+1) = 256 virtual rows. Output
                    rows 0..63 = head out, row 64 = denominator (ones
                    column in the v slab)."""
                    base = hh * 80
                    nc.tensor.matmul(
                        accs[hh][0:65, :],
                        lhsT=v_sb[:, jp, :, base:base + 65],
                        rhs=ex8[:, :, hh * SQ:(hh + 1) * SQ],
                        start=(jp == 0), stop=(jp == NKT // 2 - 1),
                        perf_mode=mybir.MatmulPerfMode.DoubleRow,
                    )

                # PV emission in bursts of 2 pairs every 4 tiles: longer
                # uninterrupted score-pair runs keep the PE's LDWEIGHTS
                # pipelining (a full-width PV matmul between score pairs
                # blocks the weight-load pull-ahead).
                pend_pv = []
                ex_cur = None
                for j in range(NKT):
                    if j == RELEASE_J:
                        flush_releases()
                    if fillers:
                        while fillers and fillers[0][0] <= j:
                            fillers.pop(0)[1]()
                    if j in TAIL_JS:
                        emit_tail_piece()
                    if j % 4 == 1 and len(pend_pv) > 4:
                        emit_pv_half(*pend_pv.pop(0))
                        emit_pv_half(*pend_pv.pop(0))
                        emit_pv_half(*pend_pv.pop(0))
                        emit_pv_half(*pend_pv.pop(0))
                    if j % 2 == 0:
                        ex_cur = exp_pool.tile([128, 2, 2 * SQ], f8,
                                               tag="exp",
                                               name=f"ex_{b}_{qc}_{j // 2}")
                    emit_scores(j, ex_cur)
                    if j % 2 == 1:
                        pend_pv.append((j // 2, ex_cur, 0))
                        pend_pv.append((j // 2, ex_cur, 1))
                for args in pend_pv:
                    emit_pv_half(*args)

                # chunk end: copy the denominator row out of PSUM, take its
                # reciprocal, and broadcast it across 64 partitions on the
                # (otherwise idle) GpSimd engine. The fused release+normalize
                # mul (acc rows 0..63 * rcp -> attT f16) is deferred into the
                # next chunk's ACT-only window.
                # (reciprocal_approx_fast is a custom-DVE op that requires
                # base-partition-0 SBUF operands, so stage via a copy.)
                for hh in range(2):
                    den = small_pool.tile([1, SQ], f32, tag="den",
                                          name=f"den_{b}_{qc}_{hh}")
                    nc.vector.tensor_copy(den[:], accs[hh][64:65, :])
                    rcp = small_pool.tile([1, SQ], f32, tag="rcp",
                                          name=f"rcp_{b}_{qc}_{hh}")
                    nc.vector.reciprocal_approx_fast(rcp[:], den[:])
                    bc = small_pool.tile([64, SQ], f32, tag="bc",
                                         name=f"bc_{b}_{qc}_{hh}")
                    nc.gpsimd.partition_broadcast(bc[:], rcp[:])

                    def rel_closure(hh=hh, bc=bc, accs=accs,
                                    attT_sb=attT_sb, q0=q0, q1=q1):
                        with nc.allow_low_precision(
                                reason="f16 raw attn weights"):
                            nc.vector.tensor_mul(
                                attT_sb[hh * 64:(hh + 1) * 64, q0:q1],
                                accs[hh][0:64, :],
                                bc[:],
                            )
                    pending_release.append(rel_closure)

            def emit_tail_piece(use_act=False):
                """Emit one out-projection piece (one [128 rows x 512 cols]
                matmul + release copy + DMA) from the pending queue. Pieces
                run one chunk late (normalization already resolved) and are
                spread across the key-tile loop so neither PE nor the copy
                engine sees a burst."""
                if not pending_pieces:
                    return
                b, qc, sti, oc = pending_pieces.pop(0)
                st = state[b]
                attT_sb = st["attT"]
                s0 = qc * SQ + sti * 128
                s1 = s0 + 128
                pso = ps_mix.tile([128, 512], f32, tag="mix",
                                  name=f"pso_{b}_{qc}_{sti}_{oc}")
                nc.tensor.matmul(
                    pso[:],
                    lhsT=attT_sb[:, s0:s1],
                    rhs=wo_sb[:, oc * 512:(oc + 1) * 512],
                    start=True, stop=True,
                )
                ob = out_pool.tile([128, 512], f16, tag="ob",
                                   name=f"ob_{b}_{qc}_{sti}_{oc}")
                with nc.allow_low_precision(reason="f16 partial out"):
                    if use_act:
                        nc.scalar.copy(ob[:], pso[:])
                    else:
                        nc.vector.tensor_copy(ob[:], pso[:])
                nc.sync.dma_start(
                    out=part[b, s0:s1, oc * 512:(oc + 1) * 512],
                    in_=ob[:],
                )

            # ---- emission schedule ----
            pending_pieces = []
            pending_release = []

            def flush_releases():
                while pending_release:
                    pending_release.pop(0)()

            def queue_tail(b, qc):
                for sti in range(SQ // 128):
                    for oc in range(D // 512):
                        pending_pieces.append((b, qc, sti, oc))

            def drain_tails(use_act=False):
                flush_releases()
                alt = False
                while pending_pieces:
                    emit_tail_piece(use_act=use_act and alt)
                    alt = not alt

            # startup: proj chunk 0, then q-chunk 0 attention with proj
            # chunks 1..7 interleaved (proj m feeds key tiles 4m..4m+3;
            # emitted at j = 4(m-1)+1 so each has a 3-tile lead).
            alloc_batch(0)
            proj_dma(0, 0)
            proj_dma(0, 1)
            proj_dma(0, 2)
            proj_mms(0, 0)

            def startup_thunk(m):
                if m + 2 < NM:
                    proj_dma(0, m + 2)
                proj_mms(0, m)

            startup = [(4 * (m - 1) + 1, (lambda m=m: startup_thunk(m)))
                       for m in range(1, NM)]
            emit_attn(0, 0, fillers=startup, all_act=True)
            queue_tail(0, 0)

            alloc_batch(1)
            proj_dma(1, 0)
            done_m = 0
            for qc in range(1, NQC):
                fill = []

                def b1_thunk(m):
                    proj_mms(1, m)
                    if m + 1 < NM:
                        proj_dma(1, m + 1)

                if done_m < NM:
                    fill.append((1, lambda m=done_m: b1_thunk(m)))
                    done_m += 1
                if qc == NQC - 1 and done_m < NM:
                    fill.append((15, lambda m=done_m: proj_mms(1, m)))
                    done_m += 1
                emit_attn(0, qc, fillers=fill)
                queue_tail(0, qc)
            if state[1] is None:
                alloc_batch(1)
            while done_m < NM:
                emit_proj_chunk(1, done_m)
                done_m += 1
            for qc in range(NQC):
                emit_attn(1, qc)
                queue_tail(1, qc)
            drain_tails(use_act=True)

    nc.compile()
    return nc


def shard_inputs(x, Wq, bq, Wk, bk, Wv, bv, Wo, bo, S=S_FULL):
    """Host-side sharding: returns list of 8 per-core input dicts."""
    x = np.asarray(x, dtype=np.float32)
    xT = np.ascontiguousarray(x.transpose(0, 2, 1)).astype(np.float16)  # [B, D, S]
    in_maps = []
    for c in range(NCORES):
        sl = slice(c * DC, (c + 1) * DC)
        in_maps.append({
            "xT": xT,
            "wqT": np.ascontiguousarray(np.asarray(Wq)[sl, :].T).astype(np.float16),
            "wkT": np.ascontiguousarray(np.asarray(Wk)[sl, :].T).astype(np.float16),
            "wvT": np.ascontiguousarray(np.asarray(Wv)[sl, :].T).astype(np.float16),
            "woT": np.ascontiguousarray(np.asarray(Wo)[:, sl].T).astype(np.float16),
            "bq": np.ascontiguousarray(np.asarray(bq)[sl], dtype=np.float32),
            "bk": np.ascontiguousarray(np.asarray(bk)[sl], dtype=np.float32),
            "ones": np.ones((128, 32), dtype=np.float16),
        })
    return in_maps


_NC_CACHE = {}


def _get_nc(S=S_FULL):
    if S not in _NC_CACHE:
        _NC_CACHE[S] = build_kernel(S)
    return _NC_CACHE[S]


def kernel(x, Wq, bq, Wk, bk, Wv, bv, Wo, bo, _trace=False, _trace_cores=None):
    from concourse import bass_utils

    nc = _get_nc(S_FULL)
    in_maps = shard_inputs(x, Wq, bq, Wk, bk, Wv, bv, Wo, bo)
    kwargs = {}
    if _trace:
        kwargs = dict(trace=True, trace_cores=_trace_cores or [0])
    res = bass_utils.run_bass_kernel_spmd(
        nc, in_maps, core_ids=list(range(NCORES)), **kwargs)
    out = np.zeros((B, S_FULL, D), dtype=np.float32)
    for c in range(NCORES):
        out += res.results[c]["part"].astype(np.float32)
    # bv is folded out of the device kernel: softmax rows sum to one, so its
    # contribution to the output is the constant Wo @ bv. Add it with bo here.
    bias = (np.asarray(Wo, dtype=np.float64) @ np.asarray(bv, dtype=np.float64)
            + np.asarray(bo, dtype=np.float64))
    out += bias.astype(np.float32)[None, None, :]
    if _trace:
        kernel._last_results = res
    return out


# revision 29
# speedup vs baseline: 1.0030x; 1.0030x over previous
"""Trainium2 Bass kernel for nn_MemoryEfficientAttention (full MHA).

Reference computation (fp32):
    q = split_heads(x @ Wq.T + bq); k, v likewise
    attn = softmax(q @ k.T / sqrt(64))
    out = merge_heads(attn @ v) @ Wo.T + bo

Shapes: B=2, S=4096, D=1024, H=16, head_dim=64.

Sharding across 8 NeuronCores (Megatron attention sharding):
  - 2 heads per core (= 128 of the 1024 projection dims, contiguous slice).
  - Q/K/V projections column-parallel, output projection row-parallel;
    the 8 per-core partial outputs are summed on the host (+ bo).
  - bv never enters the device: softmax rows sum to 1, so its entire effect
    on the output is the constant vector Wo @ bv, added on the host.

Per-core kernel (flash-attention style, nothing S^2-sized touches HBM):
  Phase 1: qT/kT = W_c @ x.T + b_c as fp16 matmuls ([128, S] transposed
           layouts); V projected directly in natural [S, 128] layout
           (x-tile stationary), stored in fp8 pair slabs with an
           interleaved ones-column per head ([vA|1|pad|vB|1]) that makes
           the PV matmul accumulate the softmax denominator in PSUM row
           64 for free.
  Phase 2: per (batch, q-chunk of 512): loop over 32 key tiles:
           scoresT[kpos, q] for both heads (row-packed in the PE array,
           they run concurrently), exp split between ScalarE (exact) and
           VectorE (Schraudolph fp8 bit-trick exp); scores run several
           key tiles ahead of the PV matmuls, which are emitted in bursts
           of 2 pairs so score-pair runs keep LDWEIGHTS pipelining. PV
           runs in fp8e4 DoubleRow: one matmul per head contracts a
           key-tile PAIR (256 virtual rows). At chunk end the denominator
           row is copied out, reciprocal'd (DVE), and broadcast across 64
           partitions on the otherwise-idle GpSimd engine
           (partition_broadcast); the PSUM release copy is FUSED with
           normalization (tensor_mul acc*rcp_bcast -> attT f16), deferred
           into the next chunk's ACT-only window. Out-projection pieces
           of the previous chunk are spread one-per-3-key-tiles.
  Phase 3: out[s, :] = attnT_c.T @ WoT_c (natural layout, clean DMA out).
  Startup: batch-0 projections are interleaved INTO q-chunk 0's key-tile
           loop (proj chunk m feeds key tiles 4m..4m+3), so ACT starts
           exp work ~4us in instead of after the whole projection phase.
"""

import sys

if "/opt/trn_rl_repo" not in sys.path:
    sys.path.insert(0, "/opt/trn_rl_repo")

import numpy as np

B = 2
S_FULL = 4096
D = 1024
H = 16
HD = 64
NCORES = 8
DC = 128          # head dims per core (2 heads x 64)
SCALE = 1.0 / 8.0  # 1/sqrt(64)

# --- softmax-exp engine split (columns of each [128, 1024] scores tile) ---
# ACT does exact exp; DVE computes a Schraudolph fp8 exp
# (i8 = floor(8*(x*SCALE*log2e + 7 + SIG) + 0.5), bit-viewed as f8e4).
# The flat half-LSB shift cancels in softmax normalization; the residual
# piecewise-linear bias (~3% max) costs ~5e-3 total rel err.
EXP_ACT = 768
EXP_DVE = 1024 - EXP_ACT
EXP_FULL_J = 4    # first key tiles per chunk: ACT does all 1024 cols, giving
                  # DVE room to clear its chunk-boundary burst off the PV path
RELEASE_J = 2     # key-tile index at which the previous chunk's deferred
                  # fused release+normalize muls are emitted on DVE
TAIL_JS = (7, 10, 13, 16, 19, 22, 25, 28)  # key-tile indices at which the
                  # previous chunk's out-projection pieces are emitted
SCH8_SIG = -0.046
SCH8_C1 = float(8.0 * np.log2(np.e) * SCALE)
SCH8_C2 = float(8.0 * (7.0 + SCH8_SIG) + 0.5)


def build_kernel(S=S_FULL):
    """Build the per-core Bass program. Returns the compiled Bacc object."""
    import concourse.bacc as bacc
    import concourse.tile as tile
    from concourse import mybir

    f32 = mybir.dt.float32
    f16 = mybir.dt.float16
    AF = mybir.ActivationFunctionType

    KT = D // 128       # k-tiles over the projection contraction dim
    SQ = 512            # q-chunk size
    NQC = S // SQ       # q chunks per batch
    NKT = S // 128      # key tiles per batch
    NM = S // 512       # x chunks for projections

    nc = bacc.Bacc("TRN2", target_bir_lowering=False, debug=False,
                   num_devices=NCORES)

    u8 = mybir.dt.uint8
    f8 = mybir.dt.float8e4

    xT = nc.dram_tensor("xT", [B, D, S], f16, kind="ExternalInput").ap()
    wqT = nc.dram_tensor("wqT", [D, DC], f16, kind="ExternalInput").ap()
    wkT = nc.dram_tensor("wkT", [D, DC], f16, kind="ExternalInput").ap()
    wvT = nc.dram_tensor("wvT", [D, DC], f16, kind="ExternalInput").ap()
    woT = nc.dram_tensor("woT", [DC, D], f16, kind="ExternalInput").ap()
    bq = nc.dram_tensor("bq", [DC], f32, kind="ExternalInput").ap()
    bk = nc.dram_tensor("bk", [DC], f32, kind="ExternalInput").ap()
    ones = nc.dram_tensor("ones", [128, 32], f16, kind="ExternalInput").ap()
    part = nc.dram_tensor("part", [B, S, D], f16, kind="ExternalOutput").ap()

    with tile.TileContext(nc) as tc:
        with (
            tc.tile_pool(name="consts", bufs=1) as consts,
            tc.tile_pool(name="xt", bufs=5) as xt_pool,
            tc.tile_pool(name="qkv", bufs=2) as qkv_pool,
            tc.tile_pool(name="exp", bufs=4) as exp_pool,
            tc.tile_pool(name="att", bufs=2) as att_pool,
            tc.tile_pool(name="small", bufs=4) as small_pool,
            tc.tile_pool(name="outs", bufs=6) as out_pool,
            tc.tile_pool(name="ps_mix", bufs=2, space="PSUM") as ps_mix,
            tc.tile_pool(name="ps_scores", bufs=2, space="PSUM") as ps_scores,
            tc.tile_pool(name="ps_acc", bufs=2, space="PSUM") as ps_acc,
        ):
            # ---- constants ----
            wq_sb = consts.tile([128, KT, DC], f16)
            wk_sb = consts.tile([128, KT, DC], f16)
            wv_sb = consts.tile([128, KT, DC], f16)
            wo_sb = consts.tile([128, D], f16)
            bq_sb = consts.tile([128, 1], f32)
            bk_sb = consts.tile([128, 1], f32)

            # tiny dummy exp so the ACT table set loads during startup DMAs
            warm = consts.tile([128, 1], f32)
            nc.vector.memset(warm[:], 0.0)
            nc.scalar.activation(warm[:], warm[:], AF.Exp, scale=1.0)

            # spread the startup weight loads across the three DMA issue
            # queues so the first projection chunk's matmuls are not gated
            # on one serial software-DGE stream
            for eng, w_sb, w_dram in ((nc.sync, wk_sb, wkT),
                                      (nc.scalar, wq_sb, wqT),
                                      (nc.sync, wv_sb, wvT)):
                eng.dma_start(
                    out=w_sb[:],
                    in_=w_dram.rearrange("(kt p) m -> p kt m", p=128),
                )
            nc.scalar.dma_start(out=wo_sb[:], in_=woT)
            for b_sb, b_dram in ((bq_sb, bq), (bk_sb, bk)):
                nc.gpsimd.dma_start(out=b_sb[:], in_=b_dram.rearrange("(p o) -> p o", o=1))

            state = [None, None]  # per-batch dict of tiles

            def alloc_batch(b):
                qT_sb = qkv_pool.tile([128, S], f16, tag="qT", name=f"qT_{b}")
                kT_sb = qkv_pool.tile([128, S], f16, tag="kT", name=f"kT_{b}")
                # v in fp8e4 DoubleRow pair layout: per key-tile PAIR jp,
                # slab c = key tile 2jp+c: [vA(64) | 1 | pad | vB(64) | 1]
                # (ones column accumulates the softmax denominator in PSUM
                # row 64 of the PV accumulator for free).
                v_sb = qkv_pool.tile([128, NKT // 2, 2, 160], f8, tag="v",
                                     name=f"v_{b}")
                nc.vector.memset(v_sb[:, :, :, 64:65], 1.0)
                nc.vector.memset(v_sb[:, :, :, 144:145], 1.0)
                attT_sb = att_pool.tile([128, S], f16, tag="attT",
                                        name=f"attT_{b}")
                state[b] = dict(qT=qT_sb, kT=kT_sb, v=v_sb, attT=attT_sb)

            xt_box = {}

            def proj_dma(b, m):
                """Issue the x-chunk load for (b, m) - emitted 1-2 chunks
                ahead of proj_mms so the 1MB DMA never stalls the PE."""
                xt = xt_pool.tile([128, KT, 512], f16, tag="xt",
                                  name=f"xt_{b}_{m}")
                xt_box[(b, m)] = xt
                xsrc = xT[b][:, m * 512:(m + 1) * 512].rearrange(
                    "(kt p) s -> p kt s", p=128)
                third = KT // 3 + 1
                nc.sync.dma_start(out=xt[:, 0:third, :],
                                  in_=xsrc[:, 0:third, :])
                nc.scalar.dma_start(out=xt[:, third:2 * third, :],
                                    in_=xsrc[:, third:2 * third, :])
                nc.gpsimd.dma_start(out=xt[:, 2 * third:, :],
                                    in_=xsrc[:, 2 * third:, :])

            def proj_mms(b, m):
                """Projection matmuls for x columns [m*512, (m+1)*512)."""
                st = state[b]
                xt = xt_box.pop((b, m))
                for w_sb, b_sb, dst in (
                    (wk_sb, bk_sb, st["kT"]),
                    (wq_sb, bq_sb, st["qT"]),
                ):
                    ps = ps_mix.tile([128, 512], f32, tag="mix",
                                      name=f"ps_{b}_{m}")
                    for j in range(KT):
                        nc.tensor.matmul(
                            ps[:],
                            lhsT=w_sb[:, j, :],
                            rhs=xt[:, j, :],
                            start=(j == 0),
                            stop=(j == KT - 1),
                        )
                    nc.vector.tensor_scalar_add(
                        dst[:, m * 512:(m + 1) * 512], ps[:], b_sb[:],
                    )
                # V in natural layout: x-tile stationary, Wv moving.
                for t in range(4):
                    psv = ps_mix.tile([128, 512], f32, tag="mix",
                                       name=f"psv_{b}_{m}_{t}")
                    for j in range(KT):
                        nc.tensor.matmul(
                            psv[:, 0:DC],
                            lhsT=xt[:, j, t * 128:(t + 1) * 128],
                            rhs=wv_sb[:, j, :],
                            start=(j == 0),
                            stop=(j == KT - 1),
                        )
                    kt_idx = m * 4 + t
                    jp, cc = kt_idx // 2, kt_idx % 2
                    with nc.allow_low_precision(reason="fp8 PV operand"):
                        nc.vector.tensor_copy(
                            state[b]["v"][:, jp, cc, 0:64], psv[:, 0:64])
                        nc.vector.tensor_copy(
                            state[b]["v"][:, jp, cc, 80:144], psv[:, 64:128])

            def emit_proj_chunk(b, m):
                proj_dma(b, m)
                proj_mms(b, m)

            def emit_attn(b, qc, fillers=None, all_act=False):
                """Attention for one q-chunk of 512 rows.

                Software-pipelined: scores for key tile j+1 are emitted
                (and thus queued on the in-order PE) BEFORE the PV matmuls
                of tile j, so the PE streams scores while ACT/DVE exp
                catches up. `fillers` is a list of (j, thunk) pairs: thunk
                is emitted when the key-tile loop reaches j (used to
                interleave projection chunks into the attention stream).
                """
                st = state[b]
                qT_sb, kT_sb, v_sb = st["qT"], st["kT"], st["v"]
                attT_sb = st["attT"]
                q0, q1 = qc * SQ, (qc + 1) * SQ
                acc_a = ps_acc.tile([128, SQ], f32, tag="acc",
                                    name=f"acca_{b}_{qc}")
                acc_b = ps_acc.tile([128, SQ], f32, tag="acc",
                                    name=f"accb_{b}_{qc}")
                accs = [acc_a, acc_b]

                if all_act:
                    full_j = NKT
                elif fillers:
                    full_j = EXP_FULL_J + 4
                else:
                    full_j = EXP_FULL_J

                def emit_scores(j, ex8):
                    """Scores pair for key tile j; exp written into slab
                    j%2 of the fp8 pair tile ex8."""
                    k0, k1 = j * 128, (j + 1) * 128
                    pss = ps_scores.tile([128, 2 * SQ], f32, tag="scores",
                                         name=f"pss_{b}_{qc}_{j}")
                    for hh in range(2):
                        nc.tensor.matmul(
                            pss[:, hh * SQ:(hh + 1) * SQ],
                            lhsT=kT_sb[hh * 64:(hh + 1) * 64, k0:k1],
                            rhs=qT_sb[hh * 64:(hh + 1) * 64, q0:q1],
                            start=True, stop=True,
                        )
                    sl = j % 2
                    with nc.allow_low_precision(reason="fp8 softmax weights"):
                        if j < full_j:
                            nc.scalar.activation(ex8[:, sl, :], pss[:],
                                                 AF.Exp, scale=SCALE)
                        else:
                            c0 = EXP_ACT
                            nc.scalar.activation(ex8[:, sl, 0:c0],
                                                 pss[:, 0:c0],
                                                 AF.Exp, scale=SCALE)
                            nc.vector.tensor_scalar(
                                ex8[:, sl, c0:].bitcast(u8), pss[:, c0:],
                                SCH8_C1, SCH8_C2,
                                op0=mybir.AluOpType.mult,
                                op1=mybir.AluOpType.add)

                def emit_pv_half(jp, ex8, hh):
                    """fp8 DoubleRow PV: one matmul per head contracts the
                    key-tile PAIR (2jp, 2jp+1) = 256 virtual rows. Output
                    rows 0..63 = head out, row 64 = denominator (ones
                    column in the v slab)."""
                    base = hh * 80
                    nc.tensor.matmul(
                        accs[hh][0:65, :],
                        lhsT=v_sb[:, jp, :, base:base + 65],
                        rhs=ex8[:, :, hh * SQ:(hh + 1) * SQ],
                        start=(jp == 0), stop=(jp == NKT // 2 - 1),
                        perf_mode=mybir.MatmulPerfMode.DoubleRow,
                    )

                # PV emission in bursts of 2 pairs every 4 tiles: longer
                # uninterrupted score-pair runs keep the PE's LDWEIGHTS
                # pipelining (a full-width PV matmul between score pairs
                # blocks the weight-load pull-ahead).
                pend_pv = []
                ex_cur = None
                for j in range(NKT):
                    if j == RELEASE_J:
                        flush_releases()
                    if fillers:
                        while fillers and fillers[0][0] <= j:
                            fillers.pop(0)[1]()
                    if j in TAIL_JS:
                        emit_tail_piece()
                    if j % 4 == 1 and len(pend_pv) > 4:
                        emit_pv_half(*pend_pv.pop(0))
                        emit_pv_half(*pend_pv.pop(0))
                        emit_pv_half(*pend_pv.pop(0))
                        emit_pv_half(*pend_pv.pop(0))
                    if j % 2 == 0:
                        ex_cur = exp_pool.tile([128, 2, 2 * SQ], f8,
                                               tag="exp",
                                               name=f"ex_{b}_{qc}_{j // 2}")
                    emit_scores(j, ex_cur)
                    if j % 2 == 1:
                        pend_pv.append((j // 2, ex_cur, 0))
                        pend_pv.append((j // 2, ex_cur, 1))
                for args in pend_pv:
                    emit_pv_half(*args)

                # chunk end: copy the denominator row out of PSUM, take its
                # reciprocal, and broadcast it across 64 partitions on the
                # (otherwise idle) GpSimd engine. The fused release+normalize
                # mul (acc rows 0..63 * rcp -> attT f16) is deferred into the
                # next chunk's ACT-only window.
                # (reciprocal_approx_fast is a custom-DVE op that requires
                # base-partition-0 SBUF operands, so stage via a copy.)
                for hh in range(2):
                    den = small_pool.tile([1, SQ], f32, tag="den",
                                          name=f"den_{b}_{qc}_{hh}")
                    nc.vector.tensor_copy(den[:], accs[hh][64:65, :])
                    rcp = small_pool.tile([1, SQ], f32, tag="rcp",
                                          name=f"rcp_{b}_{qc}_{hh}")
                    nc.vector.reciprocal_approx_fast(rcp[:], den[:])
                    bc = small_pool.tile([64, SQ], f32, tag="bc",
                                         name=f"bc_{b}_{qc}_{hh}")
                    nc.gpsimd.partition_broadcast(bc[:], rcp[:])

                    def rel_closure(hh=hh, bc=bc, accs=accs,
                                    attT_sb=attT_sb, q0=q0, q1=q1):
                        with nc.allow_low_precision(
                                reason="f16 raw attn weights"):
                            nc.vector.tensor_mul(
                                attT_sb[hh * 64:(hh + 1) * 64, q0:q1],
                                accs[hh][0:64, :],
                                bc[:],
                            )
                    pending_release.append(rel_closure)

            def emit_tail_piece(use_act=False):
                """Emit one out-projection piece (one [128 rows x 512 cols]
                matmul + release copy + DMA) from the pending queue. Pieces
                run one chunk late (normalization already resolved) and are
                spread across the key-tile loop so neither PE nor the copy
                engine sees a burst."""
                if not pending_pieces:
                    return
                b, qc, sti, oc = pending_pieces.pop(0)
                st = state[b]
                attT_sb = st["attT"]
                s0 = qc * SQ + sti * 128
                s1 = s0 + 128
                pso = ps_mix.tile([128, 512], f32, tag="mix",
                                  name=f"pso_{b}_{qc}_{sti}_{oc}")
                nc.tensor.matmul(
                    pso[:],
                    lhsT=attT_sb[:, s0:s1],
                    rhs=wo_sb[:, oc * 512:(oc + 1) * 512],
                    start=True, stop=True,
                )
                ob = out_pool.tile([128, 512], f16, tag="ob",
                                   name=f"ob_{b}_{qc}_{sti}_{oc}")
                with nc.allow_low_precision(reason="f16 partial out"):
                    if use_act:
                        nc.scalar.copy(ob[:], pso[:])
                    else:
                        nc.vector.tensor_copy(ob[:], pso[:])
                nc.sync.dma_start(
                    out=part[b, s0:s1, oc * 512:(oc + 1) * 512],
                    in_=ob[:],
                )

            # ---- emission schedule ----
            pending_pieces = []
            pending_release = []

            def flush_releases():
                while pending_release:
                    pending_release.pop(0)()

            def queue_tail(b, qc):
                for sti in range(SQ // 128):
                    for oc in range(D // 512):
                        pending_pieces.append((b, qc, sti, oc))

            def drain_tails(use_act=False):
                flush_releases()
                alt = False
                while pending_pieces:
                    emit_tail_piece(use_act=use_act and alt)
                    alt = not alt

            # startup: proj chunk 0, then q-chunk 0 attention with proj
            # chunks 1..7 interleaved (proj m feeds key tiles 4m..4m+3;
            # emitted at j = 4(m-1)+1 so each has a 3-tile lead).
            alloc_batch(0)
            proj_dma(0, 0)
            proj_dma(0, 1)
            proj_dma(0, 2)
            proj_mms(0, 0)

            def startup_thunk(m):
                if m + 2 < NM:
                    proj_dma(0, m + 2)
                proj_mms(0, m)

            startup = [(4 * (m - 1) + 1, (lambda m=m: startup_thunk(m)))
                       for m in range(1, NM)]
            emit_attn(0, 0, fillers=startup, all_act=True)
            queue_tail(0, 0)

            alloc_batch(1)
            proj_dma(1, 0)
            done_m = 0
            for qc in range(1, NQC):
                fill = []

                def b1_thunk(m):
                    proj_mms(1, m)
                    if m + 1 < NM:
                        proj_dma(1, m + 1)

                if done_m < NM:
                    fill.append((1, lambda m=done_m: b1_thunk(m)))
                    done_m += 1
                if qc == NQC - 1 and done_m < NM:
                    fill.append((15, lambda m=done_m: proj_mms(1, m)))
                    done_m += 1
                emit_attn(0, qc, fillers=fill)
                queue_tail(0, qc)
            if state[1] is None:
                alloc_batch(1)
            while done_m < NM:
                emit_proj_chunk(1, done_m)
                done_m += 1
            for qc in range(NQC):
                emit_attn(1, qc)
                queue_tail(1, qc)
            drain_tails(use_act=True)

    nc.compile()
    return nc


def shard_inputs(x, Wq, bq, Wk, bk, Wv, bv, Wo, bo, S=S_FULL):
    """Host-side sharding: returns list of 8 per-core input dicts."""
    x = np.asarray(x, dtype=np.float32)
    xT = np.ascontiguousarray(x.transpose(0, 2, 1)).astype(np.float16)  # [B, D, S]
    in_maps = []
    for c in range(NCORES):
        sl = slice(c * DC, (c + 1) * DC)
        in_maps.append({
            "xT": xT,
            "wqT": np.ascontiguousarray(np.asarray(Wq)[sl, :].T).astype(np.float16),
            "wkT": np.ascontiguousarray(np.asarray(Wk)[sl, :].T).astype(np.float16),
            "wvT": np.ascontiguousarray(np.asarray(Wv)[sl, :].T).astype(np.float16),
            "woT": np.ascontiguousarray(np.asarray(Wo)[:, sl].T).astype(np.float16),
            "bq": np.ascontiguousarray(np.asarray(bq)[sl], dtype=np.float32),
            "bk": np.ascontiguousarray(np.asarray(bk)[sl], dtype=np.float32),
            "ones": np.ones((128, 32), dtype=np.float16),
        })
    return in_maps


_NC_CACHE = {}


def _get_nc(S=S_FULL):
    if S not in _NC_CACHE:
        _NC_CACHE[S] = build_kernel(S)
    return _NC_CACHE[S]


def kernel(x, Wq, bq, Wk, bk, Wv, bv, Wo, bo, _trace=False, _trace_cores=None):
    from concourse import bass_utils

    nc = _get_nc(S_FULL)
    in_maps = shard_inputs(x, Wq, bq, Wk, bk, Wv, bv, Wo, bo)
    kwargs = {}
    if _trace:
        kwargs = dict(trace=True, trace_cores=_trace_cores or [0])
    res = bass_utils.run_bass_kernel_spmd(
        nc, in_maps, core_ids=list(range(NCORES)), **kwargs)
    out = np.zeros((B, S_FULL, D), dtype=np.float32)
    for c in range(NCORES):
        out += res.results[c]["part"].astype(np.float32)
    # bv is folded out of the device kernel: softmax rows sum to one, so its
    # contribution to the output is the constant Wo @ bv. Add it with bo here.
    bias = (np.asarray(Wo, dtype=np.float64) @ np.asarray(bv, dtype=np.float64)
            + np.asarray(bo, dtype=np.float64))
    out += bias.astype(np.float32)[None, None, :]
    if _trace:
        kernel._last_results = res
    return out
